# revision 1
# baseline (speedup 1.0000x reference)
"""
AdaptiveBertSelfAttention on 8 TRN2 NeuronCores (Bass/Tile).

Problem: B=4, S=2048, D=768, H=12 heads of dim 64.
  q = hs @ Wq.T + bq ; k = hs @ Wk.T + bk ; v = hs @ Wv.T + bv   (per-head split)
  scores = (q k^T / 8) * sf[h] + mask ; probs = softmax(scores) ; ctx = probs @ v

Sharding: core c = (batch b = c//2, head-group g = c%2). Each core computes 6
heads of one batch: fully data/tensor parallel, no collectives.

Per-core kernel layout choices:
  - host pre-transposes hs -> hsT [768, 2048] bf16, weights -> W*T [768, 384]
    bf16 (Wk/bk pre-scaled by sf/8 so scores come out pre-scaled).
  - q/k are computed transposed: qT/kT [384, 2048] (partition = head dim).
  - scores are computed transposed: S[j, i] (partition = key pos), one PSUM
    [128,1024] tile holds the pair of heads (2-head row-tiled matmuls, K=64).
  - softmax: no max subtraction needed (scores ~ N(0, 0.31)); exp on ACT
    straight out of PSUM in [128,1024] ops; additive mask handled EXACTLY by
    scaling v rows and the denominator weights with exp(mask[j]).
  - ctx^T = (vones_h)^T @ exp(S_h): vones = [v*em | em-columns] so PSUM rows
    0:63 are the unnormalized context and rows 64:95 hold the softmax
    denominator (replicated 32x). Epilogue: two 32-partition cross-quadrant
    DVE copies bring the denominator to partitions 0:63 (plain reciprocal
    ops mis-route across quadrants on HW; tensor_copy verified OK), then
    reciprocal_approx_fast + multiply + DMA out.
Output per core: ctx^T [384, 2048] f32; host transposes back (free).

Schedule: qk(pair0) prologue; v-projection groups feed just-in-time inside
attention(0)'s first sq-chunk; the next pair's qk projection groups are
metered in half-groups into the j-loop; ctx matmuls lag exp by 2 tiles and
each chunk's normalization epilogue is deferred into the next chunk.
PSUM: 4 banks scores (2x[128,1024]) + 3 ctx + 1 proj.

build_nc(repeat=N) wraps the body in a For_i hardware loop (used by the
test harness to resolve device time above the ~1.1s PJRT dispatch noise);
parts=... builds ablated variants for phase timing.
"""

import numpy as np
import ml_dtypes
from contextlib import ExitStack

import concourse.bass as bass
import concourse.tile as tile
from concourse import bacc, mybir
from concourse.bass import ts, ds
from concourse.bass_utils import run_bass_kernel_spmd

F32 = mybir.dt.float32
BF16 = mybir.dt.bfloat16
I16 = mybir.dt.int16
AF = mybir.ActivationFunctionType
ALU = mybir.AluOpType

B, S, D = 4, 2048, 768
H, HD = 12, 64
NCORES = 8
GH = 6      # heads per core
KT = 6      # contraction tiles for projections (768/128)
PAIRS = 3   # head pairs per core
SQC = 4     # query chunks of 512
JT = 16     # key tiles of 128
GD = GH * HD  # 384 = output dims per core

# Schraudolph exp in bf16 bit-space on DVE (int16(x*2^7/ln2 + SCH_B)
# bitcast to bf16) — measured NOT faster than ACT-only on HW (the DVE queue
# becomes a serialization point) and 2.5x the error, so disabled; kept for
# reference. Pointwise sigma ~1.8%, averages out over the 2048-key sum.
SCH_A = float(2.0 ** 7 / np.log(2.0))
SCH_B = 16249.25
DVE_EXP = False


def _dve_slot(j):
    return DVE_EXP and j % 3 == 2

def build_nc(repeat=1, parts="full"):
    """parts: 'dma' | 'proj' | 'scores' | 'full' — ablation for HW timing."""
    nc = bacc.Bacc("TRN2", target_bir_lowering=False, debug=False,
                   num_devices=NCORES)

    hsT = nc.dram_tensor("hsT", [D, S], BF16, kind="ExternalInput").ap()
    wqT = nc.dram_tensor("wqT", [D, GD], BF16, kind="ExternalInput").ap()
    wkT = nc.dram_tensor("wkT", [D, GD], BF16, kind="ExternalInput").ap()
    wvT = nc.dram_tensor("wvT", [D, GD], BF16, kind="ExternalInput").ap()
    bqT = nc.dram_tensor("bqT", [128, PAIRS], F32, kind="ExternalInput").ap()
    bkT = nc.dram_tensor("bkT", [128, PAIRS], F32, kind="ExternalInput").ap()
    bvr = nc.dram_tensor("bvr", [1, GD], BF16, kind="ExternalInput").ap()
    maskT = nc.dram_tensor("maskT", [128, JT], F32, kind="ExternalInput").ap()
    out = nc.dram_tensor("out", [GD, S], F32, kind="ExternalOutput").ap()

    with tile.TileContext(nc) as tc, ExitStack() as ctx:
        sb = ctx.enter_context(tc.tile_pool(name="sb", bufs=1))
        psS = ctx.enter_context(tc.tile_pool(name="psS", bufs=2, space="PSUM"))
        psC = ctx.enter_context(tc.tile_pool(name="psC", bufs=3, space="PSUM"))
        psP = ctx.enter_context(tc.tile_pool(name="psP", bufs=1, space="PSUM"))
        eP = ctx.enter_context(tc.tile_pool(name="eP", bufs=6))
        eiP = ctx.enter_context(tc.tile_pool(name="eiP", bufs=3))
        oP = ctx.enter_context(tc.tile_pool(name="oP", bufs=3))

        # persistent SBUF tensors
        hs_sb = sb.tile([128, KT, S], BF16)
        wq_sb = sb.tile([128, KT, GD], BF16)
        wk_sb = sb.tile([128, KT, GD], BF16)
        wv_sb = sb.tile([128, KT, GD], BF16)
        q_sb = sb.tile([128, PAIRS, S], BF16)
        k_sb = sb.tile([128, PAIRS, S], BF16)
        vones = sb.tile([128, JT, GH, 96], BF16)
        bq_sb = sb.tile([128, PAIRS], F32)
        bk_sb = sb.tile([128, PAIRS], F32)
        bv_sb = sb.tile([1, GD], BF16)
        ones_sb = sb.tile([1, 128], BF16)
        m_sb = sb.tile([128, JT], F32)
        em_sb = sb.tile([128, JT], F32)
        snk = (sb.tile([128, 2048], F32, name="snk")
               if parts != "full" else None)
        acw = (sb.tile([128, 512], F32, name="acw")
               if parts == "scores" else None)
        acw_n = [0]

        def consume(ap2d, width):
            """keep `ap2d` (2-dim AP) live in ablation modes via snk += ap."""
            nc.vector.tensor_tensor(snk[:, 0:width], snk[:, 0:width], ap2d,
                                    op=ALU.add)

        def finish_ablation():
            dma(out=out[0:128, :], in_=snk[:])

        dma = nc.sync.dma_start

        loop_cm = tc.For_i(0, repeat) if repeat > 1 else None
        if loop_cm is not None:
            loop_cm.__enter__()

        # ---- input loads (small first; hs in (n-major, k) chunks so the
        # first projection group can start after ~1/4 of the hs bytes) ----
        if parts != "noop":
            dma(out=m_sb[:], in_=maskT[:])
            dma(out=bq_sb[:], in_=bqT[:])
            dma(out=bk_sb[:], in_=bkT[:])
            dma(out=bv_sb[:], in_=bvr[:])
            nc.vector.memset(ones_sb[:], 1.0)
            nc.scalar.activation(em_sb[:], m_sb[:], AF.Exp)
        if parts not in ("noop", "dmahs"):
            # kT weights first: the prologue emits kT projection groups
            # before qT, and scores j0 gates the first exp
            for k in range(KT):
                dma(out=wk_sb[:, k, :], in_=wkT[ts(k, 128), :])
            for k in range(KT):
                dma(out=wq_sb[:, k, :], in_=wqT[ts(k, 128), :])
        for n in range(SQC):
            if parts not in ("noop", "dmaw"):
                for k in range(KT):
                    dma(out=hs_sb[:, k, ts(n, 512)],
                        in_=hsT[ts(k, 128), ts(n, 512)])
            if n == 0 and parts not in ("noop", "dmahs"):
                for k in range(KT):
                    dma(out=wv_sb[:, k, :], in_=wvT[ts(k, 128), :])

        if parts != "full":
            nc.vector.memset(snk[:], 0.0)
        if parts in ("dma", "dmaw", "dmahs"):
            if parts != "dmaw":
                for k in range(KT):
                    consume(hs_sb[:, k, :], S)
            if parts != "dmahs":
                for k in range(KT):
                    consume(wq_sb[:, k, :], GD)
                    consume(wk_sb[:, k, :], GD)
                    consume(wv_sb[:, k, :], GD)

        # ---- projections ----
        def qk_proj_group(wi, p, n):
            """one (q|k, pair, n-chunk) projection group."""
            (w_sb, b_sb, dst) = ((wq_sb, bq_sb, q_sb), (wk_sb, bk_sb, k_sb))[wi]
            pp = psP.tile([128, 512], F32, tag="proj", name="pp")
            for k in range(KT):
                nc.tensor.matmul(pp[:], w_sb[:, k, ts(p, 128)],
                                 hs_sb[:, k, ts(n, 512)],
                                 start=(k == 0), stop=(k == KT - 1))
            nc.vector.tensor_scalar(dst[:, p, ts(n, 512)], pp[:],
                                    b_sb[:, p:p + 1], None, op0=ALU.add)

        def v_proj_group(s):
            """v+em columns for s-tile s, all heads."""
            vp = psP.tile([128, GD], F32, tag="proj", name="vp")
            for k in range(KT):
                nc.tensor.matmul(vp[:], hs_sb[:, k, ts(s, 128)], wv_sb[:, k, :],
                                 start=(k == 0), stop=False)
            # + bias (broadcast to all 128 rows via ones (K=1) matmul)
            nc.tensor.matmul(vp[:], ones_sb[:], bv_sb[:], start=False, stop=True)
            vv = vp.rearrange("p (h d) -> p h d", h=GH)
            nc.vector.tensor_scalar(vones[:, s, :, 0:64], vv,
                                    em_sb[:, s:s + 1], None, op0=ALU.mult)
            nc.vector.tensor_scalar(vones[:, s, :, 64:96], vv[:, :, 0:32],
                                    0.0, em_sb[:, s:s + 1],
                                    op0=ALU.mult, op1=ALU.add)

        def emit_epilogue(p, sq, cA, cB):
            for hg, cps in ((0, cA), (1, cB)):
                # denominator rows live at PSUM partitions 64:96; DVE
                # 32-partition ops may cross quadrants (copy verified on
                # HW), reciprocal may not -> copy down, then recip aligned
                den = oP.tile([64, 512], F32, tag="den", name="den")
                nc.vector.tensor_copy(den[0:32, :], cps[64:96, :])
                nc.vector.tensor_copy(den[32:64, :], cps[64:96, :])
                rd = oP.tile([64, 512], F32, tag="rd", name="rd")
                nc.vector.reciprocal_approx_fast(rd[:], den[:])
                osb = oP.tile([64, 512], F32, tag="osb", name="osb")
                nc.vector.tensor_mul(osb[:], cps[0:64, :], rd[:])
                dma(out=out[ds((2 * p + hg) * 64, 64), ts(sq, 512)],
                    in_=osb[:])

        CTX_LAG = 2

        def attention(p, jhook=None):
            pend = []          # (j, eT) awaiting ctx emission
            pend_epi = []      # (sq, cA, cB) awaiting epilogue emission
            nw = 64 if parts == "cheapctx" else 512

            def drain_epilogue():
                while pend_epi:
                    esq, ecA, ecB = pend_epi.pop(0)
                    if parts == "cheapctx":
                        nc.vector.tensor_tensor(snk[0:96, 0:64],
                                                snk[0:96, 0:64],
                                                ecA[:, 0:64], op=ALU.add)
                        nc.vector.tensor_tensor(snk[0:96, 0:64],
                                                snk[0:96, 0:64],
                                                ecB[:, 0:64], op=ALU.add)
                    else:
                        emit_epilogue(p, esq, ecA, ecB)

            for sq in range(SQC):
                if parts != "scores":
                    cA = psC.tile([96, 512], F32, tag="ctx", name="cA")
                    cB = psC.tile([96, 512], F32, tag="ctx", name="cB")
                for j in range(JT):
                    Sp = psS.tile([128, 1024], F32, tag="S", name="Sp")
                    nc.tensor.matmul(Sp[:, 0:512],
                                     k_sb[0:64, p, ts(j, 128)],
                                     q_sb[0:64, p, ts(sq, 512)],
                                     start=True, stop=True,
                                     tile_position=(0, 0))
                    if parts == "norowtile":
                        nc.tensor.matmul(Sp[:, 512:1024],
                                         k_sb[0:64, p, ts(j, 128)],
                                         q_sb[0:64, p, ts(sq, 512)],
                                         start=True, stop=True,
                                         tile_position=(0, 0))
                    else:
                        nc.tensor.matmul(Sp[:, 512:1024],
                                         k_sb[64:128, p, ts(j, 128)],
                                         q_sb[64:128, p, ts(sq, 512)],
                                         start=True, stop=True,
                                         tile_position=(64, 0))
                    if parts == "scores":
                        eT = eP.tile([128, 1024], BF16, tag="e", name="eT")
                        nc.scalar.activation(eT[:], Sp[:], AF.Exp,
                                             accum_out=acw[:, acw_n[0]:acw_n[0] + 1])
                        acw_n[0] = (acw_n[0] + 1) % 512
                        eAp = eT[:]
                    elif _dve_slot(j):
                        eI = eiP.tile([128, 1024], I16, tag="ei", name="eI")
                        nc.vector.tensor_scalar(eI[:], Sp[:], SCH_A, SCH_B,
                                                op0=ALU.mult, op1=ALU.add)
                        eAp = eI[:].bitcast(BF16)
                    elif parts == "exp512":
                        eT = eP.tile([128, 1024], BF16, tag="e", name="eT")
                        nc.scalar.activation(eT[:, 0:512], Sp[:, 0:512], AF.Exp)
                        nc.scalar.activation(eT[:, 512:1024], Sp[:, 512:1024],
                                             AF.Exp)
                        eAp = eT[:]
                    else:
                        eT = eP.tile([128, 1024], BF16, tag="e", name="eT")
                        nc.scalar.activation(eT[:], Sp[:], AF.Exp)
                        eAp = eT[:]
                    if j == 2:
                        drain_epilogue()
                    if jhook is not None:
                        jhook(sq, j)
                    if parts == "scores":
                        continue
                    pend.append((j, eAp))
                    if len(pend) > CTX_LAG:
                        jj, peT = pend.pop(0)
                        nc.tensor.matmul(cA[:, 0:nw], vones[:, jj, 2 * p, :],
                                         peT[:, 0:nw],
                                         start=(jj == 0), stop=False)
                        nc.tensor.matmul(cB[:, 0:nw],
                                         vones[:, jj, 2 * p + 1, :],
                                         peT[:, 512:512 + nw],
                                         start=(jj == 0), stop=False)
                if parts == "scores":
                    continue
                while pend:
                    jj, peT = pend.pop(0)
                    nc.tensor.matmul(cA[:, 0:nw], vones[:, jj, 2 * p, :],
                                     peT[:, 0:nw],
                                     start=(jj == 0), stop=(jj == JT - 1))
                    nc.tensor.matmul(cB[:, 0:nw], vones[:, jj, 2 * p + 1, :],
                                     peT[:, 512:512 + nw],
                                     start=(jj == 0), stop=(jj == JT - 1))
                pend_epi.append((sq, cA, cB))
            drain_epilogue()

        if parts in ("proj", "projqk"):
            for p in range(PAIRS):
                for n in range(SQC):
                    qk_proj_group(1, p, n)
                    qk_proj_group(0, p, n)
            for p in range(PAIRS):
                consume(q_sb[:, p, :], S)
                consume(k_sb[:, p, :], S)
        if parts in ("proj", "projv"):
            for s in range(JT):
                v_proj_group(s)
            vfl = vones.rearrange("p a b c -> p (a b c)")
            for c in range(4):
                consume(vfl[:, ds(c * 2048, 2048)], 2048)
            consume(vfl[:, ds(8192, 1024)], 1024)
        if parts in ("noop", "dma", "dmaw", "dmahs", "proj", "projqk", "projv"):
            finish_ablation()
            if loop_cm is not None:
                loop_cm.__exit__(None, None, None)
            do_rest = False
        else:
            do_rest = True

        if do_rest:
            # emission order: qk(0) prologue; v-proj groups feed per-j inside
            # attention(0)'s first sq-chunk (ctx j needs vones[j] just-in-time)
            # next pair's qk projection groups are metered one-at-a-time
            # into the j-loop (a whole group at a chunk gap starves ACT ~2us).
            for n in range(SQC):
                qk_proj_group(1, 0, n)   # kT pair0 (scores scan all n)
            for n in range(SQC):
                qk_proj_group(0, 0, n)

        if do_rest:
            def feeder(p, work):
                """jhook: emit v groups just-in-time in sq0 (pair 0 only)
                and metered closures from `work` across remaining slots."""
                nslots = (SQC - 1) * JT if p == 0 else SQC * JT
                stride = max(1, nslots // max(1, len(work)))
                state = {"i": 0, "slot": 0}

                def hook(sq, j):
                    if p == 0 and sq == 0:
                        v_proj_group(j)
                        return
                    s = state["slot"]
                    state["slot"] += 1
                    if s % stride == stride // 2 and state["i"] < len(work):
                        work[state["i"]]()
                        state["i"] += 1
                return hook

            def qk_work(pnext):
                # each (wi, n) projection group split into two 3-MM halves
                items = []
                for wi in (1, 0):
                    for n in range(SQC):
                        box = {}

                        def h1(wi=wi, n=n, box=box):
                            (w_sb, b_sb, dst) = ((wq_sb, bq_sb, q_sb),
                                                 (wk_sb, bk_sb, k_sb))[wi]
                            pp = psP.tile([128, 512], F32, tag="proj",
                                          name="pp")
                            box["pp"] = pp
                            for k in range(3):
                                nc.tensor.matmul(pp[:],
                                                 w_sb[:, k, ts(pnext, 128)],
                                                 hs_sb[:, k, ts(n, 512)],
                                                 start=(k == 0), stop=False)

                        def h2(wi=wi, n=n, box=box):
                            (w_sb, b_sb, dst) = ((wq_sb, bq_sb, q_sb),
                                                 (wk_sb, bk_sb, k_sb))[wi]
                            pp = box["pp"]
                            for k in range(3, KT):
                                nc.tensor.matmul(pp[:],
                                                 w_sb[:, k, ts(pnext, 128)],
                                                 hs_sb[:, k, ts(n, 512)],
                                                 start=False,
                                                 stop=(k == KT - 1))
                            nc.vector.tensor_scalar(dst[:, pnext, ts(n, 512)],
                                                    pp[:],
                                                    b_sb[:, pnext:pnext + 1],
                                                    None, op0=ALU.add)
                        items.append(h1)
                        items.append(h2)
                return items

            attention(0, jhook=feeder(0, qk_work(1)))
            attention(1, jhook=feeder(1, qk_work(2)))
            attention(2)

            if parts == "cheapctx":
                finish_ablation()
            if parts == "scores":
                consume(acw[:], 512)
                vfl = vones.rearrange("p a b c -> p (a b c)")
                for c in range(4):
                    consume(vfl[:, ds(c * 2048, 2048)], 2048)
                consume(vfl[:, ds(8192, 1024)], 1024)
                finish_ablation()

            if loop_cm is not None:
                loop_cm.__exit__(None, None, None)

    nc.compile()
    return nc


_NC_CACHE = {}


def _get_nc():
    if "nc" not in _NC_CACHE:
        _NC_CACHE["nc"] = build_nc()
    return _NC_CACHE["nc"]


def make_in_maps(hidden_states, attention_mask, Wq, bq, Wk, bk, Wv, bv,
                 scaling_factors):
    hs = np.asarray(hidden_states, np.float32)
    mask = np.asarray(attention_mask, np.float32).reshape(B, S)
    Wq = np.asarray(Wq, np.float32)
    Wk = np.asarray(Wk, np.float32)
    Wv = np.asarray(Wv, np.float32)
    bq = np.asarray(bq, np.float32)
    bk = np.asarray(bk, np.float32)
    bv = np.asarray(bv, np.float32)
    sf = np.asarray(scaling_factors, np.float32)
    bf = ml_dtypes.bfloat16

    in_maps = []
    for c in range(NCORES):
        b, g = c // 2, c % 2
        sl = slice(g * GD, (g + 1) * GD)
        scale = np.repeat(sf[g * GH:(g + 1) * GH] / 8.0, HD)  # [384]
        in_maps.append({
            "hsT": np.ascontiguousarray(hs[b].T).astype(bf),
            "wqT": np.ascontiguousarray(Wq[sl].T).astype(bf),
            "wkT": np.ascontiguousarray((Wk[sl] * scale[:, None]).T).astype(bf),
            "wvT": np.ascontiguousarray(Wv[sl].T).astype(bf),
            "bqT": np.ascontiguousarray(bq[sl].reshape(PAIRS, 128).T).astype(np.float32),
            "bkT": np.ascontiguousarray((bk[sl] * scale).reshape(PAIRS, 128).T).astype(np.float32),
            "bvr": bv[sl].reshape(1, GD).astype(bf),
            "maskT": np.ascontiguousarray(mask[b].reshape(JT, 128).T).astype(np.float32),
        })
    return in_maps


def unshard(outs):
    res = np.empty((B, S, D), np.float32)
    for c in range(NCORES):
        b, g = c // 2, c % 2
        res[b, :, g * GD:(g + 1) * GD] = np.asarray(outs[c]["out"], np.float32).T
    return res


def kernel(**inputs):
    nc = _get_nc()
    in_maps = make_in_maps(**inputs)
    res = run_bass_kernel_spmd(nc, in_maps, core_ids=list(range(NCORES)))
    return unshard(res.results)


if __name__ == "__main__":
    nc = build_nc()
    print("build + compile OK")

